# revision 29
# baseline (speedup 1.0000x reference)
"""Trainium2 Bass kernel for nn_DeepSetsFunc (gnn_message_passing).

Reference computation (per set l of S=64 tokens, d=128 features):
    combined[l,j,:] = max_i( x[l,i,:] * (1 - eye)[i,j] )   # masked all-pairs max
    cm  = (relu(combined @ W1 + b1)) @ W2 + b2
    h   = (relu([x, cm] @ W3 + b3)) @ W4 + b4
    out = x + h

Sharding: data-parallel over L=256 sets across 8 cores (32 sets = 2048
tokens per core); weights replicated.

Design notes (v6):
  * The PE pipelines bf16 matmuls at 2 cols/cycle when issued
    back-to-back (~215ns per 512-col matmul) - the kernel is
    dependency/eviction-bound, not PE-bound. Everything aims at
    continuous PE issue and balanced ACT/DVE eviction load.
  * b4 enters the L4 PSUM group as a K=1 ones-matmul (b4 row x ones),
    so the residual eviction is a single tensor_tensor: osb = ps4 + x.
  * masked all-pairs max via top-2 stats per (l, d). gpsimd only runs
    add/sub/mult tensor ops (is_lt/max are Vector-only), so chains use
      comb = ne * (m1 - m2) + m2      (ne = [x < m1] in {0,1})
    with reduces + is_lt on DVE and the mult/add tail on gpsimd,
    pipelined TWO tiles ahead of the MLP.
  * Tile 0 ramp: x tile 0 DMA'd in partition halves on both HWDGE
    queues; its stats chain + L1 run in column halves so L1 starts on
    half the tile early. W3 is split x-plane (needed early) / cm-plane
    (needed late). Late x tiles ride the software DGE (gpsimd queue).
  * Per-tile PE order: L1, L3x j0-j1, L2 (contraction order 0,3,1,2 =
    h1 eviction completion order), L3x j2-j3, L3c, L4 - the L3 x-half
    matmuls cover the h1/cm eviction drains, and allocating ps3 banks
    inside the tile keeps PSUM liveness under 8 banks.
  * Tail: last tile's L4 in two column-half accumulation groups, each
    drained to its own DMA queue.
"""

import sys

for p in ("/opt/trn_rl_repo", "/root/.axon_site/_ro/trn_rl_repo"):
    if p not in sys.path:
        sys.path.insert(0, p)

import ml_dtypes
import numpy as np

import concourse.bass as bass
import concourse.mybir as mybir
import concourse.tile as tile
from concourse import bacc
from concourse.bass_utils import run_bass_kernel_spmd

# Problem shapes (hardcoded per spec).
L, S, D = 256, 64, 128
NCORES = 8
LSH = L // NCORES          # 32 sets per core
NTOK = LSH * S             # 2048 tokens per core
D4 = 4 * D                 # 512
TT = 512                   # token tile (matmul free dim); 8 sets per tile
NTT = NTOK // TT           # 4
SETS_TT = TT // S          # 8
N_WARMUP_A = 4             # PE warmups at queue start
N_WARMUP_B = 11            # PE warmups until L1(0)A can start

F32 = mybir.dt.float32
BF16 = mybir.dt.bfloat16

_AX = mybir.AxisListType
_OP = mybir.AluOpType
_AF = mybir.ActivationFunctionType

KORD = (0, 3, 1, 2)        # L2 contraction order = h1 eviction drain order


def ts(i, size):
    return bass.ts(i, size)


def build_nc() -> bass.Bass:
    nc = bacc.Bacc("TRN2", target_bir_lowering=False, debug=False)

    xt_in = nc.dram_tensor("xt", [D, NTOK], BF16, kind="ExternalInput")
    w1 = nc.dram_tensor("W1", [D, D4], BF16, kind="ExternalInput")
    b1 = nc.dram_tensor("b1", [D4], F32, kind="ExternalInput")
    w2 = nc.dram_tensor("W2", [D4, D], BF16, kind="ExternalInput")
    b2 = nc.dram_tensor("b2", [D], F32, kind="ExternalInput")
    w3 = nc.dram_tensor("W3", [2 * D, D4], BF16, kind="ExternalInput")
    b3 = nc.dram_tensor("b3", [D4], F32, kind="ExternalInput")
    w4 = nc.dram_tensor("W4", [D4, D], BF16, kind="ExternalInput")
    b4 = nc.dram_tensor("b4", [D], F32, kind="ExternalInput")
    b4r_in = nc.dram_tensor("b4r", [1, D], BF16, kind="ExternalInput")
    out = nc.dram_tensor("out", [D, NTOK], BF16, kind="ExternalOutput")

    with tile.TileContext(nc) as tc:
        with (
            tc.tile_pool(name="const", bufs=1) as constp,
            tc.tile_pool(name="big", bufs=1) as bigp,
            tc.tile_pool(name="stat", bufs=2) as statp,
            tc.tile_pool(name="work", bufs=2) as workp,
            tc.tile_pool(name="psmm", bufs=8, space="PSUM") as psmm,
        ):
            # ---- warmup + input DMAs --------------------------------------
            zz = constp.tile([128, TT], BF16)
            nc.gpsimd.memset(zz, 0.0)
            wps = psmm.tile([128, TT], F32, tag="mm", name="wps")
            for r in range(N_WARMUP_A):
                nc.tensor.matmul(wps, zz[:, :128], zz, start=True, stop=True)

            xtc = [
                bigp.tile([128, TT], BF16, name=f"xtc{i}") for i in range(NTT)
            ]
            w3s = constp.tile([128, 2, D4], BF16)  # [:,0,:]=x-rows [:,1,:]=cm
            w1s = constp.tile([128, D4], BF16)           # [d, 4d]
            w2s = constp.tile([128, 4, D], BF16)         # [k%128, k//128, d]
            w4s = constp.tile([128, 4, D], BF16)
            # sync: x0 half + W1/W2/W4; scalar: x0 half, x1, W3 planes;
            # gpsimd SWDGE: biases + late x tiles (x2, x3)
            nc.sync.dma_start(out=xtc[0][0:64, :], in_=xt_in[0:64, 0:TT])
            nc.scalar.dma_start(out=xtc[0][64:128, :], in_=xt_in[64:128, 0:TT])
            nc.sync.dma_start(out=w1s, in_=w1[:, :])
            nc.scalar.dma_start(out=xtc[1], in_=xt_in[:, ts(1, TT)])
            nc.sync.dma_start(out=w2s, in_=w2[:, :].rearrange("(c p) n -> p c n", p=128))
            nc.scalar.dma_start(out=w3s[:, 0, :], in_=w3[0:128, :])
            nc.sync.dma_start(out=w4s, in_=w4[:, :].rearrange("(c p) n -> p c n", p=128))
            nc.scalar.dma_start(out=w3s[:, 1, :], in_=w3[128:256, :])

            b1s = constp.tile([128, 4], F32)
            nc.gpsimd.dma_start(
                out=b1s.unsqueeze(2),
                in_=b1[:].rearrange("(c p) -> p c", p=128).unsqueeze(2),
            )
            b2s = constp.tile([128, 1], F32)
            nc.gpsimd.dma_start(out=b2s, in_=b2[:].unsqueeze(1))
            b3s = constp.tile([128, 4], F32)
            nc.gpsimd.dma_start(
                out=b3s.unsqueeze(2),
                in_=b3[:].rearrange("(c p) -> p c", p=128).unsqueeze(2),
            )
            # b4 rides the L4 PSUM group as a K=1 ones-matmul
            b4r_t = constp.tile([1, D], BF16, name="b4r_t")
            nc.gpsimd.dma_start(out=b4r_t, in_=b4r_in[:, :])
            ones_row = constp.tile([1, TT], BF16, name="ones_row")
            nc.gpsimd.memset(ones_row, 1.0)
            nc.gpsimd.dma_start(out=xtc[2], in_=xt_in[:, ts(2, TT)])
            nc.gpsimd.dma_start(out=xtc[3], in_=xt_in[:, ts(3, TT)])

            combs = [
                workp.tile([128, TT], BF16, tag="comb", bufs=4,
                           name=f"comb_{i}")
                for i in range(NTT)
            ]
            st = {}

            def stats_front(i):
                """m1 = rowmax(x); ne = [x < m1]  (DVE)."""
                x3 = xtc[i].rearrange("p (l s) -> p l s", s=S)
                m1 = statp.tile([128, SETS_TT], BF16, tag="m1", name=f"m1_{i}")
                nc.vector.tensor_reduce(m1, x3, axis=_AX.X, op=_OP.max)
                m1b = m1.unsqueeze(2).broadcast_to([128, SETS_TT, S])
                ne = workp.tile([128, TT], BF16, tag="ne", name=f"ne_{i}")
                ne3 = ne.rearrange("p (l s) -> p l s", s=S)
                nc.vector.tensor_tensor(ne3, x3, m1b, op=_OP.is_lt)
                st[i] = {"x3": x3, "m1": m1, "m1b": m1b, "ne": ne, "ne3": ne3}

            def stats_t2(i):
                """t2 = x * ne  (gpsimd)."""
                s = st[i]
                t2 = workp.tile([128, TT], BF16, tag="t2", name=f"t2_{i}")
                t23 = t2.rearrange("p (l s) -> p l s", s=S)
                nc.gpsimd.tensor_mul(t23, s["x3"], s["ne3"])
                s["t23"] = t23

            def stats_m2(i):
                """m2 = rowmax(t2)  (DVE)."""
                s = st[i]
                m2 = statp.tile([128, SETS_TT], BF16, tag="m2", name=f"m2_{i}")
                nc.vector.tensor_reduce(m2, s["t23"], axis=_AX.X, op=_OP.max)
                s["m2"] = m2

            def stats_tail(i):
                """comb = ne*(m1-m2) + m2  (gpsimd mult/add tail)."""
                s = st[i]
                dm = statp.tile([128, SETS_TT], BF16, tag="dm", name=f"dm_{i}")
                nc.gpsimd.tensor_tensor(dm, s["m1"], s["m2"], op=_OP.subtract)
                dmb = dm.unsqueeze(2).broadcast_to([128, SETS_TT, S])
                nc.gpsimd.tensor_mul(s["ne3"], s["ne3"], dmb)
                m2b = s["m2"].unsqueeze(2).broadcast_to([128, SETS_TT, S])
                comb3 = combs[i].rearrange("p (l s) -> p l s", s=S)
                nc.gpsimd.tensor_tensor(comb3, s["ne3"], m2b, op=_OP.add)

            def make_comb_dve_half(i, h):
                """Full chain (old max-form) on DVE for set-half h of tile i."""
                nsets = SETS_TT // 2
                csl = slice(h * nsets * S, (h + 1) * nsets * S)
                x3 = xtc[i][:, csl].rearrange("p (l s) -> p l s", s=S)
                m1 = statp.tile(
                    [128, SETS_TT], BF16, tag="m1", name=f"m1_{i}h{h}"
                )[:, :nsets]
                nc.vector.tensor_reduce(m1, x3, axis=_AX.X, op=_OP.max)
                m1b = m1.unsqueeze(2).broadcast_to([128, nsets, S])
                ne = workp.tile(
                    [128, TT], BF16, tag="ne", name=f"ne_{i}h{h}"
                )[:, : nsets * S]
                ne3 = ne.rearrange("p (l s) -> p l s", s=S)
                nc.vector.tensor_tensor(ne3, x3, m1b, op=_OP.is_lt)
                t2 = workp.tile(
                    [128, TT], BF16, tag="t2", name=f"t2_{i}h{h}"
                )[:, : nsets * S]
                t23 = t2.rearrange("p (l s) -> p l s", s=S)
                nc.vector.tensor_mul(t23, x3, ne3)
                m2 = statp.tile(
                    [128, SETS_TT], BF16, tag="m2", name=f"m2_{i}h{h}"
                )[:, :nsets]
                nc.vector.tensor_reduce(m2, t23, axis=_AX.X, op=_OP.max)
                m2b = m2.unsqueeze(2).broadcast_to([128, nsets, S])
                nc.vector.tensor_mul(ne3, ne3, m1b)
                comb3 = combs[i][:, csl].rearrange("p (l s) -> p l s", s=S)
                nc.vector.tensor_tensor(comb3, ne3, m2b, op=_OP.max)

            # ---- ramp: tile-0 chain in column halves; chain(1) front ------
            make_comb_dve_half(0, 0)
            make_comb_dve_half(0, 1)
            stats_front(1)          # DVE: m1(1), ne(1)
            stats_t2(1)             # gps: t2 = x * ne
            # chain(1) runs max-form: gps computes ne2 = ne*m1 in parallel
            # with m2; DVE finishes with comb = max(ne2, m2). Shorter serial
            # path than the select-form tail (latency matters here).
            ne2_1 = workp.tile([128, TT], BF16, tag="ne2", name="ne2_1")
            nc.gpsimd.tensor_mul(
                ne2_1.rearrange("p (l s) -> p l s", s=S),
                st[1]["ne3"], st[1]["m1b"],
            )

            for r in range(N_WARMUP_B):
                nc.tensor.matmul(wps, zz[:, :128], zz, start=True, stop=True)

            h1_cur = None
            ps1_cur = None

            def emit_h1_evicts(i, ps1, h1):
                for j in range(4):
                    if j < 3:
                        nc.scalar.activation(
                            h1[:, j, :], ps1[j], _AF.Relu,
                            bias=b1s[:, j : j + 1],
                        )
                    else:
                        nc.vector.tensor_scalar(
                            h1[:, j, :], ps1[j], b1s[:, j : j + 1], 0.0,
                            op0=_OP.add, op1=_OP.max,
                        )

            h1_cur = None
            ps1_cur = None

            def emit_h1_evicts(i, ps1, h1):
                for j in range(4):
                    if j < 3:
                        nc.scalar.activation(
                            h1[:, j, :], ps1[j], _AF.Relu,
                            bias=b1s[:, j : j + 1],
                        )
                    else:
                        nc.vector.tensor_scalar(
                            h1[:, j, :], ps1[j], b1s[:, j : j + 1], 0.0,
                            op0=_OP.add, op1=_OP.max,
                        )

            for tt_i in range(NTT):
                cs = ts(tt_i, TT)
                last = tt_i == NTT - 1
                c = tt_i + 2            # stats chain prepared two tiles ahead

                if tt_i == 0:
                    # ---- L1(0): column halves so it starts as soon as half
                    # the tile-0 stats chain lands -------------------------
                    h1_cur = workp.tile([128, 4, TT], BF16, tag="h1",
                                        name="h1_0")
                    ps1_cur = [
                        psmm.tile([128, TT], F32, tag="mm", name=f"ps1_0_{j}")
                        for j in range(4)
                    ]
                    for hh in range(2):
                        hsl = slice(hh * 256, (hh + 1) * 256)
                        for j in range(4):
                            nc.tensor.matmul(
                                ps1_cur[j][:, hsl], w1s[:, ts(j, 128)],
                                combs[0][:, hsl], start=True, stop=True,
                            )
                    emit_h1_evicts(0, ps1_cur, h1_cur)
                    stats_m2(1)       # DVE (t2(1) landed on gps pre-loop)
                if c < NTT:
                    stats_front(c)    # DVE: m1(c), ne(c)
                # ---- L3 x-half j0/j1: covers the h1 eviction drain -------
                if tt_i == 1:
                    ps3 = ps3_t1      # filled during the tile-0 tail
                else:
                    ps3 = [
                        psmm.tile([128, TT], F32, tag="mm",
                                  name=f"ps3_{tt_i}_{j}")
                        for j in range(4)
                    ]
                    for j in range(2):
                        nc.tensor.matmul(
                            ps3[j], w3s[:, 0, ts(j, 128)], xtc[tt_i],
                            start=True, stop=False,
                        )
                # ---- L2: cm = W2.T @ h1 + b2 (k-order = eviction order) --
                ps2 = psmm.tile([128, TT], F32, tag="mm")
                for ki, k in enumerate(KORD):
                    nc.tensor.matmul(
                        ps2, w2s[:, k, :], h1_cur[:, k, :],
                        start=(ki == 0), stop=(ki == 3),
                    )
                cm = workp.tile([128, TT], BF16, tag="cm")
                nc.scalar.activation(cm, ps2, _AF.Identity, bias=b2s)
                # ---- L3 x-half j2/j3: covers the cm eviction -------------
                if tt_i != 1:
                    for j in range(2, 4):
                        nc.tensor.matmul(
                            ps3[j], w3s[:, 0, ts(j, 128)], xtc[tt_i],
                            start=True, stop=False,
                        )
                if c < NTT:
                    stats_t2(c)       # gps
                # ---- L3 cm-half + bias/relu (evictions alternate) --------
                h3 = workp.tile([128, 4, TT], BF16, tag="h3")
                for j in range(4):
                    nc.tensor.matmul(
                        ps3[j], w3s[:, 1, ts(j, 128)], cm,
                        start=False, stop=True,
                    )
                    if j % 2 == 0:
                        nc.scalar.activation(
                            h3[:, j, :], ps3[j], _AF.Relu,
                            bias=b3s[:, j : j + 1],
                        )
                    else:
                        nc.vector.tensor_scalar(
                            h3[:, j, :], ps3[j], b3s[:, j : j + 1], 0.0,
                            op0=_OP.add, op1=_OP.max,
                        )
                    if tt_i == 0 and j == 1:
                        # comb(1) = max(ne2, m2) on DVE, slotted between the
                        # tile-0 h3 evictions
                        m2b_1 = st[1]["m2"].unsqueeze(2).broadcast_to(
                            [128, SETS_TT, S]
                        )
                        nc.vector.tensor_tensor(
                            combs[1].rearrange("p (l s) -> p l s", s=S),
                            ne2_1.rearrange("p (l s) -> p l s", s=S),
                            m2b_1, op=_OP.max,
                        )
                # ---- next tile's L1 j0/j1: fills the h3 eviction drain ---
                h1_next = None
                ps1_next = None
                if tt_i == 0:
                    # tile-1 L3 x-half banks, filled interleaved with L4(0)
                    ps3_t1 = [
                        psmm.tile([128, TT], F32, tag="mm", name=f"ps3_1_{j}")
                        for j in range(4)
                    ]
                elif not last:
                    h1_next = workp.tile([128, 4, TT], BF16, tag="h1",
                                         name=f"h1_{tt_i + 1}")
                    ps1_next = [
                        psmm.tile([128, TT], F32, tag="mm",
                                  name=f"ps1_{tt_i + 1}_{j}")
                        for j in range(4)
                    ]
                # ---- L4: out = W4.T @ h3 + b4 + x ------------------------
                # b4 enters the PSUM group as a K=1 matmul (b4 row x ones);
                # the residual x rides the eviction tensor_tensor.
                if not last:
                    ps4 = psmm.tile([128, TT], F32, tag="mm")
                    for k in range(4):
                        nc.tensor.matmul(
                            ps4, w4s[:, k, :], h3[:, k, :],
                            start=(k == 0), stop=False,
                        )
                        if tt_i == 0:
                            nc.tensor.matmul(
                                ps3_t1[k], w3s[:, 0, ts(k, 128)], xtc[1],
                                start=True, stop=False,
                            )
                        elif not last and k < 2:
                            # next tile's L1 j0/j1 woven between the L4
                            # contractions: fills the h3 eviction waits
                            nc.tensor.matmul(
                                ps1_next[k], w1s[:, ts(k, 128)],
                                combs[tt_i + 1], start=True, stop=True,
                            )
                    nc.tensor.matmul(
                        ps4, b4r_t, ones_row, start=False, stop=True
                    )
                    osb = workp.tile([128, TT], BF16, tag="osb")
                    nc.vector.tensor_tensor(osb, ps4, xtc[tt_i], op=_OP.add)
                    dma_eng = nc.sync if tt_i % 2 == 0 else nc.scalar
                    dma_eng.dma_start(out=out[:, cs], in_=osb)
                    # next tile's L1 (j2/j3 for pipelined tiles; all four
                    # for tile 1, whose comb lands late) + evictions
                    if tt_i == 0:
                        h1_next = workp.tile([128, 4, TT], BF16, tag="h1",
                                             name="h1_1")
                        ps1_next = [
                            psmm.tile([128, TT], F32, tag="mm",
                                      name=f"ps1_1_{j}")
                            for j in range(4)
                        ]
                        for j in range(4):
                            nc.tensor.matmul(
                                ps1_next[j], w1s[:, ts(j, 128)], combs[1],
                                start=True, stop=True,
                            )
                    else:
                        for j in range(2, 4):
                            nc.tensor.matmul(
                                ps1_next[j], w1s[:, ts(j, 128)],
                                combs[tt_i + 1], start=True, stop=True,
                            )
                    emit_h1_evicts(tt_i + 1, ps1_next, h1_next)
                    h1_cur, ps1_cur = h1_next, ps1_next
                else:
                    # final tile: two column-half accumulation groups so the
                    # tail drains while the second half is still on the PE
                    osb = workp.tile([128, TT], BF16, tag="osb")
                    for h in range(2):
                        hsl = slice(h * 256, (h + 1) * 256)
                        ps4 = psmm.tile(
                            [128, 256], F32, tag="mm", name=f"ps4h{h}"
                        )
                        for k in range(4):
                            nc.tensor.matmul(
                                ps4, w4s[:, k, :], h3[:, k, hsl],
                                start=(k == 0), stop=False,
                            )
                        nc.tensor.matmul(
                            ps4, b4r_t, ones_row[:, hsl],
                            start=False, stop=True,
                        )
                        nc.vector.tensor_tensor(
                            osb[:, hsl], ps4, xtc[tt_i][:, hsl], op=_OP.add
                        )
                        dma_eng = nc.sync if h == 0 else nc.scalar
                        dma_eng.dma_start(
                            out=out[:, tt_i * TT + h * 256:
                                    tt_i * TT + (h + 1) * 256],
                            in_=osb[:, hsl],
                        )
                if c < NTT:
                    stats_m2(c)       # DVE, end of block
                    stats_tail(c)     # gps (runs early next block)

    nc.compile()
    return nc


_NC_CACHE = None


def make_in_maps(inputs: dict) -> list[dict]:
    f32 = np.float32
    bf = ml_dtypes.bfloat16
    x = np.asarray(inputs["set_input"], dtype=f32)
    shared = {
        "W1": np.ascontiguousarray(inputs["W1"], f32).astype(bf),
        "W2": np.ascontiguousarray(inputs["W2"], f32).astype(bf),
        "W3": np.ascontiguousarray(inputs["W3"], f32).astype(bf),
        "W4": np.ascontiguousarray(inputs["W4"], f32).astype(bf),
        "b1": np.ascontiguousarray(inputs["b1"], f32),
        "b2": np.ascontiguousarray(inputs["b2"], f32),
        "b3": np.ascontiguousarray(inputs["b3"], f32),
        "b4": np.ascontiguousarray(inputs["b4"], f32),
        "b4r": np.ascontiguousarray(inputs["b4"], f32).astype(bf).reshape(1, -1),
    }
    in_maps = []
    for c in range(NCORES):
        shard_t = np.ascontiguousarray(
            x[c * LSH : (c + 1) * LSH].reshape(NTOK, D).T
        ).astype(bf)
        in_maps.append({"xt": shard_t, **shared})
    return in_maps


def kernel(**inputs) -> np.ndarray:
    global _NC_CACHE
    if _NC_CACHE is None:
        _NC_CACHE = build_nc()
    nc = _NC_CACHE

    in_maps = make_in_maps(inputs)
    res = run_bass_kernel_spmd(nc, in_maps, core_ids=list(range(NCORES)))
    outs = [
        res.results[c]["out"].astype(np.float32).T.reshape(LSH, S, D)
        for c in range(NCORES)
    ]
    return np.concatenate(outs, axis=0)


# revision 30
# speedup vs baseline: 1.0077x; 1.0077x over previous
"""Trainium2 Bass kernel for nn_DeepSetsFunc (gnn_message_passing).

Reference computation (per set l of S=64 tokens, d=128 features):
    combined[l,j,:] = max_i( x[l,i,:] * (1 - eye)[i,j] )   # masked all-pairs max
    cm  = (relu(combined @ W1 + b1)) @ W2 + b2
    h   = (relu([x, cm] @ W3 + b3)) @ W4 + b4
    out = x + h

Sharding: data-parallel over L=256 sets across 8 cores (32 sets = 2048
tokens per core); weights replicated.

Design notes (v6):
  * The PE pipelines bf16 matmuls at 2 cols/cycle when issued
    back-to-back (~215ns per 512-col matmul) - the kernel is
    dependency/eviction-bound, not PE-bound. Everything aims at
    continuous PE issue and balanced ACT/DVE eviction load.
  * b4 enters the L4 PSUM group as a K=1 ones-matmul (b4 row x ones),
    so the residual eviction is a single tensor_tensor: osb = ps4 + x.
  * masked all-pairs max via top-2 stats per (l, d). gpsimd only runs
    add/sub/mult tensor ops (is_lt/max are Vector-only), so chains use
      comb = ne * (m1 - m2) + m2      (ne = [x < m1] in {0,1})
    with reduces + is_lt on DVE and the mult/add tail on gpsimd,
    pipelined TWO tiles ahead of the MLP.
  * Tile 0 ramp: x tile 0 DMA'd in partition halves on both HWDGE
    queues; its stats chain + L1 run in column halves so L1 starts on
    half the tile early. W3 is split x-plane (needed early) / cm-plane
    (needed late). Late x tiles ride the software DGE (gpsimd queue).
  * Per-tile PE order: L1, L3x j0-j1, L2 (contraction order 0,3,1,2 =
    h1 eviction completion order), L3x j2-j3, L3c, L4 - the L3 x-half
    matmuls cover the h1/cm eviction drains, and allocating ps3 banks
    inside the tile keeps PSUM liveness under 8 banks.
  * Tail: last tile's L4 in two column-half accumulation groups, each
    drained to its own DMA queue.
"""

import sys

for p in ("/opt/trn_rl_repo", "/root/.axon_site/_ro/trn_rl_repo"):
    if p not in sys.path:
        sys.path.insert(0, p)

import ml_dtypes
import numpy as np

import concourse.bass as bass
import concourse.mybir as mybir
import concourse.tile as tile
from concourse import bacc
from concourse.bass_utils import run_bass_kernel_spmd

# Problem shapes (hardcoded per spec).
L, S, D = 256, 64, 128
NCORES = 8
LSH = L // NCORES          # 32 sets per core
NTOK = LSH * S             # 2048 tokens per core
D4 = 4 * D                 # 512
TT = 512                   # token tile (matmul free dim); 8 sets per tile
NTT = NTOK // TT           # 4
SETS_TT = TT // S          # 8
N_WARMUP_A = 4             # PE warmups at queue start
N_WARMUP_B = 11            # PE warmups until L1(0)A can start

F32 = mybir.dt.float32
BF16 = mybir.dt.bfloat16

_AX = mybir.AxisListType
_OP = mybir.AluOpType
_AF = mybir.ActivationFunctionType

KORD = (0, 3, 1, 2)        # L2 contraction order = h1 eviction drain order


def ts(i, size):
    return bass.ts(i, size)


def build_nc() -> bass.Bass:
    nc = bacc.Bacc("TRN2", target_bir_lowering=False, debug=False)

    xt_in = nc.dram_tensor("xt", [D, NTOK], BF16, kind="ExternalInput")
    w1 = nc.dram_tensor("W1", [D, D4], BF16, kind="ExternalInput")
    b1 = nc.dram_tensor("b1", [D4], F32, kind="ExternalInput")
    w2 = nc.dram_tensor("W2", [D4, D], BF16, kind="ExternalInput")
    b2 = nc.dram_tensor("b2", [D], F32, kind="ExternalInput")
    w3 = nc.dram_tensor("W3", [2 * D, D4], BF16, kind="ExternalInput")
    b3 = nc.dram_tensor("b3", [D4], F32, kind="ExternalInput")
    w4 = nc.dram_tensor("W4", [D4, D], BF16, kind="ExternalInput")
    b4 = nc.dram_tensor("b4", [D], F32, kind="ExternalInput")
    b4r_in = nc.dram_tensor("b4r", [1, D], BF16, kind="ExternalInput")
    out = nc.dram_tensor("out", [D, NTOK], BF16, kind="ExternalOutput")

    with tile.TileContext(nc) as tc:
        with (
            tc.tile_pool(name="const", bufs=1) as constp,
            tc.tile_pool(name="big", bufs=1) as bigp,
            tc.tile_pool(name="stat", bufs=2) as statp,
            tc.tile_pool(name="work", bufs=2) as workp,
            tc.tile_pool(name="psmm", bufs=8, space="PSUM") as psmm,
        ):
            # ---- warmup + input DMAs --------------------------------------
            zz = constp.tile([128, TT], BF16)
            nc.gpsimd.memset(zz, 0.0)
            wps = psmm.tile([128, TT], F32, tag="mm", name="wps")
            for r in range(N_WARMUP_A):
                nc.tensor.matmul(wps, zz[:, :128], zz, start=True, stop=True)

            xtc = [
                bigp.tile([128, TT], BF16, name=f"xtc{i}") for i in range(NTT)
            ]
            w3s = constp.tile([128, 2, D4], BF16)  # [:,0,:]=x-rows [:,1,:]=cm
            w1s = constp.tile([128, D4], BF16)           # [d, 4d]
            w2s = constp.tile([128, 4, D], BF16)         # [k%128, k//128, d]
            w4s = constp.tile([128, 4, D], BF16)
            # sync: x0 half + W1/W2/W4; scalar: x0 half, x1, W3 planes;
            # gpsimd SWDGE: biases + late x tiles (x2, x3)
            nc.sync.dma_start(out=xtc[0][0:64, :], in_=xt_in[0:64, 0:TT])
            nc.scalar.dma_start(out=xtc[0][64:128, :], in_=xt_in[64:128, 0:TT])
            nc.sync.dma_start(out=w1s, in_=w1[:, :])
            nc.scalar.dma_start(out=xtc[1], in_=xt_in[:, ts(1, TT)])
            nc.sync.dma_start(out=w2s, in_=w2[:, :].rearrange("(c p) n -> p c n", p=128))
            nc.scalar.dma_start(out=w3s[:, 0, :], in_=w3[0:128, :])
            nc.sync.dma_start(out=w4s, in_=w4[:, :].rearrange("(c p) n -> p c n", p=128))
            nc.scalar.dma_start(out=w3s[:, 1, :], in_=w3[128:256, :])

            b1s = constp.tile([128, 4], F32)
            nc.gpsimd.dma_start(
                out=b1s.unsqueeze(2),
                in_=b1[:].rearrange("(c p) -> p c", p=128).unsqueeze(2),
            )
            b2s = constp.tile([128, 1], F32)
            nc.gpsimd.dma_start(out=b2s, in_=b2[:].unsqueeze(1))
            b3s = constp.tile([128, 4], F32)
            nc.gpsimd.dma_start(
                out=b3s.unsqueeze(2),
                in_=b3[:].rearrange("(c p) -> p c", p=128).unsqueeze(2),
            )
            # b4 rides the L4 PSUM group as a K=1 ones-matmul
            b4r_t = constp.tile([1, D], BF16, name="b4r_t")
            nc.gpsimd.dma_start(out=b4r_t, in_=b4r_in[:, :])
            ones_row = constp.tile([1, TT], BF16, name="ones_row")
            nc.gpsimd.memset(ones_row, 1.0)
            nc.gpsimd.dma_start(out=xtc[2], in_=xt_in[:, ts(2, TT)])
            nc.gpsimd.dma_start(out=xtc[3], in_=xt_in[:, ts(3, TT)])

            combs = [
                workp.tile([128, TT], BF16, tag="comb", bufs=4,
                           name=f"comb_{i}")
                for i in range(NTT)
            ]
            st = {}

            def stats_front(i):
                """m1 = rowmax(x); ne = [x < m1]  (DVE)."""
                x3 = xtc[i].rearrange("p (l s) -> p l s", s=S)
                m1 = statp.tile([128, SETS_TT], BF16, tag="m1", name=f"m1_{i}")
                nc.vector.tensor_reduce(m1, x3, axis=_AX.X, op=_OP.max)
                m1b = m1.unsqueeze(2).broadcast_to([128, SETS_TT, S])
                ne = workp.tile([128, TT], BF16, tag="ne", name=f"ne_{i}")
                ne3 = ne.rearrange("p (l s) -> p l s", s=S)
                nc.vector.tensor_tensor(ne3, x3, m1b, op=_OP.is_lt)
                st[i] = {"x3": x3, "m1": m1, "m1b": m1b, "ne": ne, "ne3": ne3}

            def stats_t2(i):
                """t2 = x * ne  (gpsimd)."""
                s = st[i]
                t2 = workp.tile([128, TT], BF16, tag="t2", name=f"t2_{i}")
                t23 = t2.rearrange("p (l s) -> p l s", s=S)
                nc.gpsimd.tensor_mul(t23, s["x3"], s["ne3"])
                s["t23"] = t23

            def stats_m2(i):
                """m2 = rowmax(t2)  (DVE)."""
                s = st[i]
                m2 = statp.tile([128, SETS_TT], BF16, tag="m2", name=f"m2_{i}")
                nc.vector.tensor_reduce(m2, s["t23"], axis=_AX.X, op=_OP.max)
                s["m2"] = m2

            def stats_tail(i):
                """comb = ne*(m1-m2) + m2  (gpsimd mult/add tail)."""
                s = st[i]
                dm = statp.tile([128, SETS_TT], BF16, tag="dm", name=f"dm_{i}")
                nc.gpsimd.tensor_tensor(dm, s["m1"], s["m2"], op=_OP.subtract)
                dmb = dm.unsqueeze(2).broadcast_to([128, SETS_TT, S])
                nc.gpsimd.tensor_mul(s["ne3"], s["ne3"], dmb)
                m2b = s["m2"].unsqueeze(2).broadcast_to([128, SETS_TT, S])
                comb3 = combs[i].rearrange("p (l s) -> p l s", s=S)
                nc.gpsimd.tensor_tensor(comb3, s["ne3"], m2b, op=_OP.add)

            def make_comb_dve_half(i, h):
                """Full chain (old max-form) on DVE for set-half h of tile i."""
                nsets = SETS_TT // 2
                csl = slice(h * nsets * S, (h + 1) * nsets * S)
                x3 = xtc[i][:, csl].rearrange("p (l s) -> p l s", s=S)
                m1 = statp.tile(
                    [128, SETS_TT], BF16, tag="m1", name=f"m1_{i}h{h}"
                )[:, :nsets]
                nc.vector.tensor_reduce(m1, x3, axis=_AX.X, op=_OP.max)
                m1b = m1.unsqueeze(2).broadcast_to([128, nsets, S])
                ne = workp.tile(
                    [128, TT], BF16, tag="ne", name=f"ne_{i}h{h}"
                )[:, : nsets * S]
                ne3 = ne.rearrange("p (l s) -> p l s", s=S)
                nc.vector.tensor_tensor(ne3, x3, m1b, op=_OP.is_lt)
                t2 = workp.tile(
                    [128, TT], BF16, tag="t2", name=f"t2_{i}h{h}"
                )[:, : nsets * S]
                t23 = t2.rearrange("p (l s) -> p l s", s=S)
                nc.vector.tensor_mul(t23, x3, ne3)
                m2 = statp.tile(
                    [128, SETS_TT], BF16, tag="m2", name=f"m2_{i}h{h}"
                )[:, :nsets]
                nc.vector.tensor_reduce(m2, t23, axis=_AX.X, op=_OP.max)
                m2b = m2.unsqueeze(2).broadcast_to([128, nsets, S])
                nc.vector.tensor_mul(ne3, ne3, m1b)
                comb3 = combs[i][:, csl].rearrange("p (l s) -> p l s", s=S)
                nc.vector.tensor_tensor(comb3, ne3, m2b, op=_OP.max)

            # ---- ramp: tile-0 chain in column halves; chain(1) front ------
            make_comb_dve_half(0, 0)
            make_comb_dve_half(0, 1)
            stats_front(1)          # DVE: m1(1), ne(1)
            stats_t2(1)             # gps: t2 = x * ne
            # chain(1) runs max-form: gps computes ne2 = ne*m1 in parallel
            # with m2; DVE finishes with comb = max(ne2, m2). Shorter serial
            # path than the select-form tail (latency matters here).
            ne2_1 = workp.tile([128, TT], BF16, tag="ne2", name="ne2_1")
            nc.gpsimd.tensor_mul(
                ne2_1.rearrange("p (l s) -> p l s", s=S),
                st[1]["ne3"], st[1]["m1b"],
            )

            for r in range(N_WARMUP_B):
                nc.tensor.matmul(wps, zz[:, :128], zz, start=True, stop=True)

            h1_cur = None
            ps1_cur = None

            def emit_h1_evicts(i, ps1, h1):
                for j in range(4):
                    if j < 3:
                        nc.scalar.activation(
                            h1[:, j, :], ps1[j], _AF.Relu,
                            bias=b1s[:, j : j + 1],
                        )
                    else:
                        nc.vector.tensor_scalar(
                            h1[:, j, :], ps1[j], b1s[:, j : j + 1], 0.0,
                            op0=_OP.add, op1=_OP.max,
                        )

            h1_cur = None
            ps1_cur = None

            def emit_h1_evicts(i, ps1, h1):
                for j in range(4):
                    if j < 3:
                        nc.scalar.activation(
                            h1[:, j, :], ps1[j], _AF.Relu,
                            bias=b1s[:, j : j + 1],
                        )
                    else:
                        nc.vector.tensor_scalar(
                            h1[:, j, :], ps1[j], b1s[:, j : j + 1], 0.0,
                            op0=_OP.add, op1=_OP.max,
                        )

            for tt_i in range(NTT):
                cs = ts(tt_i, TT)
                last = tt_i == NTT - 1
                c = tt_i + 2            # stats chain prepared two tiles ahead

                if tt_i == 0:
                    # ---- L1(0): column halves so it starts as soon as half
                    # the tile-0 stats chain lands -------------------------
                    h1_cur = workp.tile([128, 4, TT], BF16, tag="h1",
                                        name="h1_0")
                    ps1_cur = [
                        psmm.tile([128, TT], F32, tag="mm", name=f"ps1_0_{j}")
                        for j in range(4)
                    ]
                    for hh in range(2):
                        hsl = slice(hh * 256, (hh + 1) * 256)
                        for j in range(4):
                            nc.tensor.matmul(
                                ps1_cur[j][:, hsl], w1s[:, ts(j, 128)],
                                combs[0][:, hsl], start=True, stop=True,
                            )
                    emit_h1_evicts(0, ps1_cur, h1_cur)
                    stats_m2(1)       # DVE (t2(1) landed on gps pre-loop)
                if c < NTT:
                    stats_front(c)    # DVE: m1(c), ne(c)
                # ---- L3 x-half j0/j1: covers the h1 eviction drain -------
                if tt_i == 1:
                    ps3 = ps3_t1      # filled during the tile-0 tail
                else:
                    ps3 = [
                        psmm.tile([128, TT], F32, tag="mm",
                                  name=f"ps3_{tt_i}_{j}")
                        for j in range(4)
                    ]
                    for j in range(2):
                        nc.tensor.matmul(
                            ps3[j], w3s[:, 0, ts(j, 128)], xtc[tt_i],
                            start=True, stop=False,
                        )
                # ---- L2: cm = W2.T @ h1 + b2 (k-order = eviction order) --
                ps2 = psmm.tile([128, TT], F32, tag="mm")
                for ki, k in enumerate(KORD):
                    nc.tensor.matmul(
                        ps2, w2s[:, k, :], h1_cur[:, k, :],
                        start=(ki == 0), stop=(ki == 3),
                    )
                cm = workp.tile([128, TT], BF16, tag="cm")
                nc.scalar.activation(cm, ps2, _AF.Identity, bias=b2s)
                # ---- L3 x-half j2/j3: covers the cm eviction -------------
                if tt_i != 1:
                    for j in range(2, 4):
                        nc.tensor.matmul(
                            ps3[j], w3s[:, 0, ts(j, 128)], xtc[tt_i],
                            start=True, stop=False,
                        )
                if c < NTT:
                    stats_t2(c)       # gps
                # ---- L3 cm-half + bias/relu (evictions alternate) --------
                h3 = workp.tile([128, 4, TT], BF16, tag="h3")
                for j in range(4):
                    nc.tensor.matmul(
                        ps3[j], w3s[:, 1, ts(j, 128)], cm,
                        start=False, stop=True,
                    )
                    if j % 2 == 0:
                        nc.scalar.activation(
                            h3[:, j, :], ps3[j], _AF.Relu,
                            bias=b3s[:, j : j + 1],
                        )
                    else:
                        nc.vector.tensor_scalar(
                            h3[:, j, :], ps3[j], b3s[:, j : j + 1], 0.0,
                            op0=_OP.add, op1=_OP.max,
                        )
                    if tt_i == 0 and j == 1:
                        # comb(1) = max(ne2, m2) on DVE, slotted between the
                        # tile-0 h3 evictions
                        m2b_1 = st[1]["m2"].unsqueeze(2).broadcast_to(
                            [128, SETS_TT, S]
                        )
                        nc.vector.tensor_tensor(
                            combs[1].rearrange("p (l s) -> p l s", s=S),
                            ne2_1.rearrange("p (l s) -> p l s", s=S),
                            m2b_1, op=_OP.max,
                        )
                # ---- next tile's L1 j0/j1: fills the h3 eviction drain ---
                h1_next = None
                ps1_next = None
                if tt_i == 0:
                    # tile-1 L3 x-half banks, filled interleaved with L4(0)
                    ps3_t1 = [
                        psmm.tile([128, TT], F32, tag="mm", name=f"ps3_1_{j}")
                        for j in range(4)
                    ]
                elif not last:
                    h1_next = workp.tile([128, 4, TT], BF16, tag="h1",
                                         name=f"h1_{tt_i + 1}")
                    ps1_next = [
                        psmm.tile([128, TT], F32, tag="mm",
                                  name=f"ps1_{tt_i + 1}_{j}")
                        for j in range(4)
                    ]
                    for j in range(2):
                        nc.tensor.matmul(
                            ps1_next[j], w1s[:, ts(j, 128)], combs[tt_i + 1],
                            start=True, stop=True,
                        )
                # ---- L4: out = W4.T @ h3 + b4 + x ------------------------
                # b4 enters the PSUM group as a K=1 matmul (b4 row x ones);
                # the residual x rides the eviction tensor_tensor.
                if not last:
                    ps4 = psmm.tile([128, TT], F32, tag="mm")
                    for k in range(4):
                        nc.tensor.matmul(
                            ps4, w4s[:, k, :], h3[:, k, :],
                            start=(k == 0), stop=False,
                        )
                        if tt_i == 0:
                            nc.tensor.matmul(
                                ps3_t1[k], w3s[:, 0, ts(k, 128)], xtc[1],
                                start=True, stop=False,
                            )
                    nc.tensor.matmul(
                        ps4, b4r_t, ones_row, start=False, stop=True
                    )
                    osb = workp.tile([128, TT], BF16, tag="osb")
                    nc.vector.tensor_tensor(osb, ps4, xtc[tt_i], op=_OP.add)
                    dma_eng = nc.sync if tt_i % 2 == 0 else nc.scalar
                    dma_eng.dma_start(out=out[:, cs], in_=osb)
                    # next tile's L1 (j2/j3 for pipelined tiles; all four
                    # for tile 1, whose comb lands late) + evictions
                    if tt_i == 0:
                        h1_next = workp.tile([128, 4, TT], BF16, tag="h1",
                                             name="h1_1")
                        ps1_next = [
                            psmm.tile([128, TT], F32, tag="mm",
                                      name=f"ps1_1_{j}")
                            for j in range(4)
                        ]
                        for j in range(4):
                            nc.tensor.matmul(
                                ps1_next[j], w1s[:, ts(j, 128)], combs[1],
                                start=True, stop=True,
                            )
                    else:
                        for j in range(2, 4):
                            nc.tensor.matmul(
                                ps1_next[j], w1s[:, ts(j, 128)],
                                combs[tt_i + 1], start=True, stop=True,
                            )
                    emit_h1_evicts(tt_i + 1, ps1_next, h1_next)
                    h1_cur, ps1_cur = h1_next, ps1_next
                else:
                    # final tile: two column-half accumulation groups so the
                    # tail drains while the second half is still on the PE
                    osb = workp.tile([128, TT], BF16, tag="osb")
                    for h in range(2):
                        hsl = slice(h * 256, (h + 1) * 256)
                        ps4 = psmm.tile(
                            [128, 256], F32, tag="mm", name=f"ps4h{h}"
                        )
                        for k in range(4):
                            nc.tensor.matmul(
                                ps4, w4s[:, k, :], h3[:, k, hsl],
                                start=(k == 0), stop=False,
                            )
                        nc.tensor.matmul(
                            ps4, b4r_t, ones_row[:, hsl],
                            start=False, stop=True,
                        )
                        nc.vector.tensor_tensor(
                            osb[:, hsl], ps4, xtc[tt_i][:, hsl], op=_OP.add
                        )
                        dma_eng = nc.sync if h == 0 else nc.scalar
                        dma_eng.dma_start(
                            out=out[:, tt_i * TT + h * 256:
                                    tt_i * TT + (h + 1) * 256],
                            in_=osb[:, hsl],
                        )
                if c < NTT:
                    stats_m2(c)       # DVE, end of block
                    stats_tail(c)     # gps (runs early next block)

    nc.compile()
    return nc


_NC_CACHE = None


def make_in_maps(inputs: dict) -> list[dict]:
    f32 = np.float32
    bf = ml_dtypes.bfloat16
    x = np.asarray(inputs["set_input"], dtype=f32)
    shared = {
        "W1": np.ascontiguousarray(inputs["W1"], f32).astype(bf),
        "W2": np.ascontiguousarray(inputs["W2"], f32).astype(bf),
        "W3": np.ascontiguousarray(inputs["W3"], f32).astype(bf),
        "W4": np.ascontiguousarray(inputs["W4"], f32).astype(bf),
        "b1": np.ascontiguousarray(inputs["b1"], f32),
        "b2": np.ascontiguousarray(inputs["b2"], f32),
        "b3": np.ascontiguousarray(inputs["b3"], f32),
        "b4": np.ascontiguousarray(inputs["b4"], f32),
        "b4r": np.ascontiguousarray(inputs["b4"], f32).astype(bf).reshape(1, -1),
    }
    in_maps = []
    for c in range(NCORES):
        shard_t = np.ascontiguousarray(
            x[c * LSH : (c + 1) * LSH].reshape(NTOK, D).T
        ).astype(bf)
        in_maps.append({"xt": shard_t, **shared})
    return in_maps


def kernel(**inputs) -> np.ndarray:
    global _NC_CACHE
    if _NC_CACHE is None:
        _NC_CACHE = build_nc()
    nc = _NC_CACHE

    in_maps = make_in_maps(inputs)
    res = run_bass_kernel_spmd(nc, in_maps, core_ids=list(range(NCORES)))
    outs = [
        res.results[c]["out"].astype(np.float32).T.reshape(LSH, S, D)
        for c in range(NCORES)
    ]
    return np.concatenate(outs, axis=0)
